# revision 1
# baseline (speedup 1.0000x reference)
"""GraphSAGE (3-layer, mean aggregator) on 8 Trainium2 NeuronCores.

Strategy: dst-shard nodes across 8 cores (12544 each, degree-sorted within
core so per-block work is uniform across cores -> one SPMD program).
Aggregation: dma_gather of x[src] (edge-major, 4x 32768-row chunk tables for
int16 indices) + PE matmul against per-tile one-hot masks built on DVE
((iota == dstrel) * 1/deg, mean folded in) accumulated in PSUM per
(chunk, block), evicted into a feature-major SBUF accumulator. Dense phase:
feature-major W.T @ xT matmuls, ACT bias+relu, PE transpose to row-major,
AllGather to build the next layer's gather table.
"""

import numpy as np

N = 100000
NEDGE = 1600000
DIN = 117
D = 128
NLAYER = 3
NCORE = 8
BLK = 128
NBLK = 98
SH = BLK * NBLK          # 12544 nodes per core
NT = SH * NCORE          # 100352 table rows
CH = 32768               # gather chunk rows (int16 index limit)
NCHUNK = 4
CALL = 1024             # gather slots per dma_gather call
ORIG_SH = N // NCORE     # 12500 real nodes per core

_CACHE = {}
TRACE = False
LAST_RESULT = None


def _preprocess(src, dst):
    """Host-side graph preprocessing. Returns the static plan + per-core arrays."""
    deg = np.bincount(dst, minlength=N)

    # permutation: per original core range, sort by degree desc; perm[new] = orig
    perm = np.full(NT, -1, np.int64)
    for c in range(NCORE):
        orig = np.arange(c * ORIG_SH, (c + 1) * ORIG_SH)
        order = np.argsort(-deg[orig], kind="stable")
        perm[c * SH : c * SH + ORIG_SH] = orig[order]
    real = perm >= 0
    inv = np.empty(N, np.int64)
    inv[perm[real]] = np.flatnonzero(real)

    s_n = inv[src]          # permuted src id [0, NT)
    d_n = inv[dst]
    core = d_n // SH
    chunk = s_n // CH
    block = (d_n % SH) // BLK

    # counts per (core, chunk, block); static regions R = max over cores
    key = (core * NCHUNK + chunk) * NBLK + block
    cnt = np.bincount(key, minlength=NCORE * NCHUNK * NBLK).reshape(
        NCORE, NCHUNK, NBLK
    )
    R = cnt.max(axis=0)                      # [NCHUNK, NBLK]
    chunk_len = R.sum(axis=1)
    chunk_tot = ((chunk_len + 127) // 128) * 128
    chunk_off = np.concatenate([[0], np.cumsum(chunk_tot)])[:NCHUNK]
    reg_off = np.zeros((NCHUNK, NBLK), np.int64)
    for s in range(NCHUNK):
        reg_off[s] = chunk_off[s] + np.concatenate([[0], np.cumsum(R[s])[:-1]])
    nslot = int(chunk_off[-1] + chunk_tot[-1])

    # gather call grid (static): per chunk, windows of CALL slots
    calls = []  # (chunk, slot0, n)
    for s in range(NCHUNK):
        p = int(chunk_off[s])
        end = p + int(chunk_tot[s])
        while p < end:
            n = min(CALL, end - p)
            calls.append((s, p, n))
            p += n

    # matmul entries (static): (s, b, tile, start, stop) in stream order
    entries = []
    ev_first = np.full(NBLK, -1)  # first nonempty chunk per block -> copy
    for s in range(NCHUNK):
        for b in range(NBLK):
            if R[s, b] == 0:
                continue
            if ev_first[b] < 0:
                ev_first[b] = s
            t0 = int(reg_off[s, b]) // 128
            t1 = int(-(-(reg_off[s, b] + R[s, b]) // 128))
            for t in range(t0, t1):
                entries.append((s, b, t, t == t0, t == t1 - 1))
    nent = len(entries)

    # per-core slot arrays
    deg_new = np.bincount(d_n, minlength=NT).astype(np.float64)
    w_new = 1.0 / np.maximum(deg_new, 1.0)

    idx_all = np.zeros((NCORE, nslot), np.int64)      # chunk-local src index
    slot_dn = np.full((NCORE, nslot), -(10 ** 6), np.int64)
    slot_w = np.zeros((NCORE, nslot), np.float32)
    slot_real = np.zeros((NCORE, nslot), bool)
    for c in range(NCORE):
        m = core == c
        sc, dc, bc, cc = s_n[m], d_n[m], block[m], chunk[m]
        k = cc * NBLK + bc
        order = np.argsort(k, kind="stable")
        ks = k[order]
        # offset within group
        grp_start = np.searchsorted(ks, np.arange(NCHUNK * NBLK))
        within = np.arange(len(ks)) - grp_start[ks]
        pos = reg_off[(ks // NBLK), (ks % NBLK)] + within
        idx_all[c, pos] = sc[order] % CH
        slot_dn[c, pos] = dc[order]
        slot_w[c, pos] = w_new[dc[order]].astype(np.float32)
        slot_real[c, pos] = True

    # pads keep idx=0 (read a real in-chunk row; excluded by mask dstrel=-1)

    # idx wrapped [16, nslot/16] replicated to 128 partitions
    idxw = np.zeros((NCORE, 128, nslot // 16), np.int16)
    for c in range(NCORE):
        wrap = idx_all[c].reshape(nslot // 16, 16).T.astype(np.int16)
        idxw[c] = np.tile(wrap, (8, 1))

    # per-entry dstrel / w columns [128, nent]
    dstrel = np.full((NCORE, 128, nent), -1.0, np.float32)
    wcol = np.zeros((NCORE, 128, nent), np.float32)
    for i, (s, b, t, _, _) in enumerate(entries):
        sl = slice(t * 128, (t + 1) * 128)
        for c in range(NCORE):
            rel = (slot_dn[c, sl] % SH) - b * BLK
            rel = np.where(slot_dn[c, sl] < 0, -1, rel)
            dstrel[c, :, i] = rel.astype(np.float32)
            wcol[c, :, i] = slot_w[c, sl]

    plan = {
        "calls": calls,
        "entries": entries,
        "nslot": nslot,
        "nent": nent,
        "ev_first": ev_first,
        "R": R,
    }
    data = {
        "perm": perm,
        "idxw": idxw,
        "dstrel": dstrel,
        "wcol": wcol,
    }
    return plan, data


def _build(plan):
    import concourse.bass as bass
    import concourse.bacc as bacc
    import concourse.mybir as mybir
    import concourse.tile as tile
    from concourse import library_config

    f32 = mybir.dt.float32
    nc = bacc.Bacc("TRN2", target_bir_lowering=False)

    nslot, nent = plan["nslot"], plan["nent"]
    calls, entries = plan["calls"], plan["entries"]
    ev_first = plan["ev_first"]

    # I/O
    h0t = nc.dram_tensor("h0t", [DIN, SH], f32, kind="ExternalInput")
    idxs = nc.dram_tensor("idxs", [128, nslot // 16], mybir.dt.int16, kind="ExternalInput")
    dstrel_d = nc.dram_tensor("dstrel", [128, nent], f32, kind="ExternalInput")
    wcol_d = nc.dram_tensor("wcol", [128, nent], f32, kind="ExternalInput")
    iota_d = nc.dram_tensor("iota", [128, 128], f32, kind="ExternalInput")
    ident_d = nc.dram_tensor("ident", [128, 128], f32, kind="ExternalInput")
    win_d = nc.dram_tensor("win", [DIN, D], f32, kind="ExternalInput")
    bin_d = nc.dram_tensor("bin", [128, 1], f32, kind="ExternalInput")
    ws_d = nc.dram_tensor("ws", [D, NLAYER * D], f32, kind="ExternalInput")
    wn_d = nc.dram_tensor("wn", [D, NLAYER * D], f32, kind="ExternalInput")
    bsage_d = nc.dram_tensor("bsage", [128, NLAYER], f32, kind="ExternalInput")
    out_d = nc.dram_tensor("out", [SH, D], f32, kind="ExternalOutput")

    # internal DRAM: shard stage + gather tables
    shard = nc.dram_tensor("shard", [SH, D], f32)
    tables = [
        nc.dram_tensor(f"table{l}", [NT, D], f32, addr_space="Shared")
        for l in range(NLAYER)
    ]
    rg = [list(range(NCORE))]

    with tile.TileContext(nc) as tc:
        with (
            tc.tile_pool(name="big", bufs=1) as big,
            tc.tile_pool(name="wpool", bufs=1) as wp,
            tc.tile_pool(name="piece", bufs=3) as piecep,
            tc.tile_pool(name="mask", bufs=4) as maskp,
            tc.tile_pool(name="orm", bufs=2) as ormp,
            tc.tile_pool(name="agg", bufs=4, space="PSUM") as aggp,
            tc.tile_pool(name="dns", bufs=2, space="PSUM") as dnsp,
            tc.tile_pool(name="tps", bufs=2, space="PSUM") as tpsp,
        ):
            nc.gpsimd.load_library(library_config.mlp)

            # persistent SBUF
            acc = big.tile([128, SH], f32, tag="acc")
            xT = big.tile([128, SH], f32, tag="xT")
            dstrel_t = big.tile([128, nent], f32, tag="dstrel")
            wcol_t = big.tile([128, nent], f32, tag="wcol")
            iota_t = wp.tile([128, 128], f32, tag="iota")
            ident_t = wp.tile([128, 128], f32, tag="ident")
            win_t = wp.tile([DIN, D], f32, tag="win")
            bin_t = wp.tile([128, 1], f32, tag="bin")
            ws_t = wp.tile([D, NLAYER * D], f32, tag="ws")
            wn_t = wp.tile([D, NLAYER * D], f32, tag="wn")
            bsage_t = wp.tile([128, NLAYER], f32, tag="bsage")

            nc.sync.dma_start(out=dstrel_t[:], in_=dstrel_d[:])
            nc.sync.dma_start(out=wcol_t[:], in_=wcol_d[:])
            nc.sync.dma_start(out=iota_t[:], in_=iota_d[:])
            nc.sync.dma_start(out=ident_t[:], in_=ident_d[:])
            nc.sync.dma_start(out=win_t[:], in_=win_d[:])
            nc.sync.dma_start(out=bin_t[:], in_=bin_d[:])
            nc.sync.dma_start(out=ws_t[:], in_=ws_d[:])
            nc.sync.dma_start(out=wn_t[:], in_=wn_d[:])
            nc.sync.dma_start(out=bsage_t[:], in_=bsage_d[:])

            def out_block(src_fm, b, dram, base):
                """src_fm: [128 feat, 128 dst] SBUF -> transpose -> dram rows."""
                ps = tpsp.tile([128, 128], f32, tag="tp")
                nc.tensor.transpose(out=ps[:], in_=src_fm, identity=ident_t[:])
                orm = ormp.tile([128, 128], f32, tag="orm")
                nc.vector.tensor_copy(out=orm[:], in_=ps[:])
                nc.sync.dma_start(
                    out=dram[base + b * BLK : base + (b + 1) * BLK, :], in_=orm[:]
                )

            # ---- layer 0: xT = tanh(W_in.T @ h0T + b_in), write shard+table0
            H0G = 8
            h0piece = {}
            for b in range(NBLK):
                g, r = divmod(b, H0G)
                if r == 0:
                    nb = min(H0G, NBLK - g * H0G)
                    h0p = piecep.tile([DIN, H0G * BLK], f32, tag="h0p", name="h0p")
                    nc.sync.dma_start(
                        out=h0p[:, : nb * BLK],
                        in_=h0t[:, g * H0G * BLK : (g * H0G + nb) * BLK],
                    )
                    h0piece[g] = h0p
                ps = dnsp.tile([128, 128], f32, tag="dns")
                nc.tensor.matmul(
                    out=ps[:],
                    lhsT=win_t[:],
                    rhs=h0piece[g][:, r * BLK : (r + 1) * BLK],
                    start=True,
                    stop=True,
                )
                nc.scalar.activation(
                    out=xT[:, b * BLK : (b + 1) * BLK],
                    in_=ps[:],
                    func=mybir.ActivationFunctionType.Tanh,
                    bias=bin_t[:],
                )
                out_block(xT[:, b * BLK : (b + 1) * BLK], b, shard, 0)
            nc.gpsimd.collective_compute(
                "AllGather",
                mybir.AluOpType.bypass,
                ins=[shard[:]],
                outs=[tables[0][:]],
                replica_groups=rg,
            )

            # ---- GNN layers
            for l in range(NLAYER):
                table = tables[l]
                # aggregation: gather calls + mask matmuls in stream order
                piece_of_slot = {}
                for (s, p0, n) in calls:
                    ix = maskp.tile([128, CALL // 16], mybir.dt.int16, tag="ix", name="ix")
                    nc.sync.dma_start(
                        out=ix[:, : n // 16], in_=idxs[:, p0 // 16 : (p0 + n) // 16]
                    )
                    pc = piecep.tile([128, CALL // 128, 128], f32, tag="piece")
                    nc.gpsimd.dma_gather(
                        pc[:, : n // 128, :],
                        table[s * CH :, :],
                        ix[:, : n // 16],
                        n,
                        n,
                        D,
                    )
                    for t in range(p0 // 128, (p0 + n) // 128):
                        piece_of_slot[t] = (pc, t - p0 // 128)

                ps_cur = {}
                MG = 16
                mk_cur = None
                for i, (s, b, t, st, sp) in enumerate(entries):
                    pc, tl = piece_of_slot[t]
                    gi, ri = divmod(i, MG)
                    if ri == 0:
                        ng = min(MG, nent - gi * MG)
                        mk_cur = maskp.tile([128, MG, 128], f32, tag="mask", name="mk")
                        iota_b = bass.AP(
                            iota_t.tensor,
                            iota_t[:].offset,
                            [list(iota_t[:].ap[0]), [0, ng], list(iota_t[:].ap[1])],
                        )
                        dsl_b = dstrel_t[:, gi * MG : gi * MG + ng].to_broadcast(
                            [128, ng, 128]
                        )
                        w_b = wcol_t[:, gi * MG : gi * MG + ng].to_broadcast(
                            [128, ng, 128]
                        )
                        nc.vector.tensor_tensor(
                            out=mk_cur[:, :ng, :],
                            in0=iota_b,
                            in1=dsl_b,
                            op=mybir.AluOpType.is_equal,
                        )
                        nc.vector.tensor_tensor(
                            out=mk_cur[:, :ng, :],
                            in0=mk_cur[:, :ng, :],
                            in1=w_b,
                            op=mybir.AluOpType.mult,
                        )
                    if st:
                        ps_cur[b] = aggp.tile([128, 128], f32, tag="agg", name="aggps")
                    nc.tensor.matmul(
                        out=ps_cur[b][:],
                        lhsT=pc[:, tl, :],
                        rhs=mk_cur[:, ri, :],
                        start=st,
                        stop=sp,
                    )
                    if sp:
                        dsl = acc[:, b * BLK : (b + 1) * BLK]
                        if ev_first[b] == s:
                            nc.vector.tensor_copy(out=dsl, in_=ps_cur[b][:])
                        else:
                            nc.vector.tensor_add(out=dsl, in0=dsl, in1=ps_cur[b][:])

                # dense phase
                last = l == NLAYER - 1
                dram = out_d if last else shard
                for b in range(NBLK):
                    bsl = slice(b * BLK, (b + 1) * BLK)
                    ps = dnsp.tile([128, 128], f32, tag="dns")
                    nc.tensor.matmul(
                        out=ps[:],
                        lhsT=ws_t[:, l * D : (l + 1) * D],
                        rhs=xT[:, bsl],
                        start=True,
                        stop=False,
                    )
                    nc.tensor.matmul(
                        out=ps[:],
                        lhsT=wn_t[:, l * D : (l + 1) * D],
                        rhs=acc[:, bsl],
                        start=False,
                        stop=True,
                    )
                    nc.scalar.activation(
                        out=xT[:, bsl],
                        in_=ps[:],
                        func=mybir.ActivationFunctionType.Relu,
                        bias=bsage_t[:, l : l + 1],
                    )
                    out_block(xT[:, bsl], b, dram, 0)
                if not last:
                    nc.gpsimd.collective_compute(
                        "AllGather",
                        mybir.AluOpType.bypass,
                        ins=[shard[:]],
                        outs=[tables[l + 1][:]],
                        replica_groups=rg,
                    )

    nc.compile()
    return nc


def kernel(h0, src, dst, W_in, b_in, W_self, W_neigh, b_sage):
    from concourse.bass_utils import run_bass_kernel_spmd

    h0 = np.asarray(h0)
    src = np.asarray(src)
    dst = np.asarray(dst)
    key = "k"
    if key not in _CACHE:
        plan, data = _preprocess(src, dst)
        nc = _build(plan)
        _CACHE[key] = (plan, data, nc)
    plan, data, nc = _CACHE[key]
    perm = data["perm"]

    # permuted h0 (virtual rows zero), feature-major per core
    h0p = np.zeros((NT, DIN), np.float32)
    real = perm >= 0
    h0p[real] = h0[perm[real]]

    bin_col = np.zeros((128, 1), np.float32)
    bin_col[:D, 0] = b_in
    bsage_col = np.zeros((128, NLAYER), np.float32)
    bsage_col[:D, :] = np.asarray(b_sage).T
    iota = np.tile(np.arange(128, dtype=np.float32), (128, 1))
    ident = np.eye(128, dtype=np.float32)
    ws = np.concatenate([np.asarray(W_self)[l] for l in range(NLAYER)], axis=1).astype(np.float32)
    wn = np.concatenate([np.asarray(W_neigh)[l] for l in range(NLAYER)], axis=1).astype(np.float32)

    in_maps = []
    for c in range(NCORE):
        in_maps.append(
            {
                "h0t": np.ascontiguousarray(h0p[c * SH : (c + 1) * SH].T),
                "idxs": data["idxw"][c],
                "dstrel": data["dstrel"][c],
                "wcol": data["wcol"][c],
                "iota": iota,
                "ident": ident,
                "win": np.asarray(W_in, np.float32),
                "bin": bin_col,
                "ws": ws,
                "wn": wn,
                "bsage": bsage_col,
            }
        )

    global LAST_RESULT
    res = run_bass_kernel_spmd(
        nc, in_maps, core_ids=list(range(NCORE)), trace=TRACE
    )
    LAST_RESULT = res

    out = np.empty((N, D), np.float32)
    for c in range(NCORE):
        o = res.results[c]["out"]
        pc = perm[c * SH : (c + 1) * SH]
        m = pc >= 0
        out[pc[m]] = o[m]
    return out



# revision 3
# speedup vs baseline: 1.3716x; 1.3716x over previous
"""GraphSAGE (3-layer, mean aggregator) on 8 Trainium2 NeuronCores.

Strategy: dst-shard nodes across 8 cores (12544 each, degree-sorted within
core so per-block work is uniform across cores -> one SPMD program).
Aggregation: dma_gather of x[src] (edge-major, 4x 32768-row chunk tables for
int16 indices) + PE matmul against per-tile one-hot masks built on DVE
((iota == dstrel) * 1/deg, mean folded in) accumulated in PSUM per
(chunk, block), evicted into a feature-major SBUF accumulator. Dense phase:
feature-major W.T @ xT matmuls, ACT bias+relu, PE transpose to row-major,
AllGather to build the next layer's gather table.
"""

import numpy as np

N = 100000
NEDGE = 1600000
DIN = 117
D = 128
NLAYER = 3
NCORE = 8
BLK = 128
NBLK = 98
SH = BLK * NBLK          # 12544 nodes per core
NT = SH * NCORE          # 100352 table rows
CH = 32768               # gather chunk rows (int16 index limit)
NCHUNK = 4
CALL = 1024             # gather slots per dma_gather call
ORIG_SH = N // NCORE     # 12500 real nodes per core

_CACHE = {}
TRACE = False
LAST_RESULT = None


def _preprocess(src, dst):
    """Host-side graph preprocessing. Returns the static plan + per-core arrays."""
    deg = np.bincount(dst, minlength=N)

    # permutation: per original core range, sort by degree desc; perm[new] = orig
    perm = np.full(NT, -1, np.int64)
    for c in range(NCORE):
        orig = np.arange(c * ORIG_SH, (c + 1) * ORIG_SH)
        order = np.argsort(-deg[orig], kind="stable")
        perm[c * SH : c * SH + ORIG_SH] = orig[order]
    real = perm >= 0
    inv = np.empty(N, np.int64)
    inv[perm[real]] = np.flatnonzero(real)

    s_n = inv[src]          # permuted src id [0, NT)
    d_n = inv[dst]
    core = d_n // SH
    chunk = s_n // CH
    block = (d_n % SH) // BLK

    # counts per (core, chunk, block); static regions R = max over cores
    key = (core * NCHUNK + chunk) * NBLK + block
    cnt = np.bincount(key, minlength=NCORE * NCHUNK * NBLK).reshape(
        NCORE, NCHUNK, NBLK
    )
    R = cnt.max(axis=0)                      # [NCHUNK, NBLK]
    chunk_len = R.sum(axis=1)
    chunk_tot = ((chunk_len + 127) // 128) * 128
    chunk_off = np.concatenate([[0], np.cumsum(chunk_tot)])[:NCHUNK]
    reg_off = np.zeros((NCHUNK, NBLK), np.int64)
    for s in range(NCHUNK):
        reg_off[s] = chunk_off[s] + np.concatenate([[0], np.cumsum(R[s])[:-1]])
    nslot = int(chunk_off[-1] + chunk_tot[-1])

    # gather call grid (static): per chunk, windows of CALL slots
    calls = []  # (chunk, slot0, n)
    for s in range(NCHUNK):
        p = int(chunk_off[s])
        end = p + int(chunk_tot[s])
        while p < end:
            n = min(CALL, end - p)
            calls.append((s, p, n))
            p += n

    # matmul entries (static): (s, b, tile, start, stop) in stream order
    entries = []
    ev_first = np.full(NBLK, -1)  # first nonempty chunk per block -> copy
    for s in range(NCHUNK):
        for b in range(NBLK):
            if R[s, b] == 0:
                continue
            if ev_first[b] < 0:
                ev_first[b] = s
            t0 = int(reg_off[s, b]) // 128
            t1 = int(-(-(reg_off[s, b] + R[s, b]) // 128))
            for t in range(t0, t1):
                entries.append((s, b, t, t == t0, t == t1 - 1))
    nent = len(entries)

    # per-core slot arrays
    deg_new = np.bincount(d_n, minlength=NT).astype(np.float64)
    w_new = 1.0 / np.maximum(deg_new, 1.0)

    idx_all = np.zeros((NCORE, nslot), np.int64)      # chunk-local src index
    slot_dn = np.full((NCORE, nslot), -(10 ** 6), np.int64)
    slot_w = np.zeros((NCORE, nslot), np.float32)
    slot_real = np.zeros((NCORE, nslot), bool)
    for c in range(NCORE):
        m = core == c
        sc, dc, bc, cc = s_n[m], d_n[m], block[m], chunk[m]
        k = cc * NBLK + bc
        order = np.argsort(k, kind="stable")
        ks = k[order]
        # offset within group
        grp_start = np.searchsorted(ks, np.arange(NCHUNK * NBLK))
        within = np.arange(len(ks)) - grp_start[ks]
        pos = reg_off[(ks // NBLK), (ks % NBLK)] + within
        idx_all[c, pos] = sc[order] % CH
        slot_dn[c, pos] = dc[order]
        slot_w[c, pos] = w_new[dc[order]].astype(np.float32)
        slot_real[c, pos] = True

    # pads keep idx=0 (read a real in-chunk row; excluded by mask dstrel=-1)

    # idx wrapped [16, nslot/16] replicated to 128 partitions
    idxw = np.zeros((NCORE, 128, nslot // 16), np.int16)
    for c in range(NCORE):
        wrap = idx_all[c].reshape(nslot // 16, 16).T.astype(np.int16)
        idxw[c] = np.tile(wrap, (8, 1))

    # per-entry dstrel / w columns [128, nent]
    dstrel = np.full((NCORE, 128, nent), -1.0, np.float32)
    wcol = np.zeros((NCORE, 128, nent), np.float32)
    for i, (s, b, t, _, _) in enumerate(entries):
        sl = slice(t * 128, (t + 1) * 128)
        for c in range(NCORE):
            rel = (slot_dn[c, sl] % SH) - b * BLK
            rel = np.where(slot_dn[c, sl] < 0, -1, rel)
            dstrel[c, :, i] = rel.astype(np.float32)
            wcol[c, :, i] = slot_w[c, sl]

    plan = {
        "calls": calls,
        "entries": entries,
        "nslot": nslot,
        "nent": nent,
        "ev_first": ev_first,
        "R": R,
    }
    data = {
        "perm": perm,
        "idxw": idxw,
        "dstrel": dstrel,
        "wcol": wcol,
    }
    return plan, data


def _build(plan):
    import concourse.bass as bass
    import concourse.bacc as bacc
    import concourse.mybir as mybir
    import concourse.tile as tile
    from concourse import library_config

    f32 = mybir.dt.float32
    nc = bacc.Bacc("TRN2", target_bir_lowering=False, num_swdge_queues=4)

    nslot, nent = plan["nslot"], plan["nent"]
    calls, entries = plan["calls"], plan["entries"]
    ev_first = plan["ev_first"]

    # I/O
    h0t = nc.dram_tensor("h0t", [DIN, SH], f32, kind="ExternalInput")
    idxs = nc.dram_tensor("idxs", [128, nslot // 16], mybir.dt.int16, kind="ExternalInput")
    dstrel_d = nc.dram_tensor("dstrel", [128, nent], f32, kind="ExternalInput")
    wcol_d = nc.dram_tensor("wcol", [128, nent], f32, kind="ExternalInput")
    iota_d = nc.dram_tensor("iota", [128, 128], f32, kind="ExternalInput")
    ident_d = nc.dram_tensor("ident", [128, 128], f32, kind="ExternalInput")
    win_d = nc.dram_tensor("win", [DIN, D], f32, kind="ExternalInput")
    bin_d = nc.dram_tensor("bin", [128, 1], f32, kind="ExternalInput")
    ws_d = nc.dram_tensor("ws", [D, NLAYER * D], f32, kind="ExternalInput")
    wn_d = nc.dram_tensor("wn", [D, NLAYER * D], f32, kind="ExternalInput")
    bsage_d = nc.dram_tensor("bsage", [128, NLAYER], f32, kind="ExternalInput")
    out_d = nc.dram_tensor("out", [SH, D], f32, kind="ExternalOutput")

    # internal DRAM: shard stage + gather tables
    shard = nc.dram_tensor("shard", [SH, D], f32)
    tables = [
        nc.dram_tensor(f"table{l}", [NT, D], f32, addr_space="Shared")
        for l in range(NLAYER)
    ]
    rg = [list(range(NCORE))]

    with tile.TileContext(nc) as tc:
        with (
            tc.tile_pool(name="big", bufs=1) as big,
            tc.tile_pool(name="wpool", bufs=1) as wp,
            tc.tile_pool(name="piece", bufs=3) as piecep,
            tc.tile_pool(name="mask", bufs=4) as maskp,
            tc.tile_pool(name="orm", bufs=2) as ormp,
            tc.tile_pool(name="agg", bufs=4, space="PSUM") as aggp,
            tc.tile_pool(name="dns", bufs=2, space="PSUM") as dnsp,
            tc.tile_pool(name="tps", bufs=2, space="PSUM") as tpsp,
        ):
            nc.gpsimd.load_library(library_config.mlp)

            # persistent SBUF
            acc = big.tile([128, SH], f32, tag="acc")
            xT = big.tile([128, SH], f32, tag="xT")
            dstrel_t = big.tile([128, nent], f32, tag="dstrel")
            wcol_t = big.tile([128, nent], f32, tag="wcol")
            iota_t = wp.tile([128, 128], f32, tag="iota")
            ident_t = wp.tile([128, 128], f32, tag="ident")
            win_t = wp.tile([DIN, D], f32, tag="win")
            bin_t = wp.tile([128, 1], f32, tag="bin")
            ws_t = wp.tile([D, NLAYER * D], f32, tag="ws")
            wn_t = wp.tile([D, NLAYER * D], f32, tag="wn")
            bsage_t = wp.tile([128, NLAYER], f32, tag="bsage")

            nc.sync.dma_start(out=dstrel_t[:], in_=dstrel_d[:])
            nc.sync.dma_start(out=wcol_t[:], in_=wcol_d[:])
            nc.sync.dma_start(out=iota_t[:], in_=iota_d[:])
            nc.sync.dma_start(out=ident_t[:], in_=ident_d[:])
            nc.sync.dma_start(out=win_t[:], in_=win_d[:])
            nc.sync.dma_start(out=bin_t[:], in_=bin_d[:])
            nc.sync.dma_start(out=ws_t[:], in_=ws_d[:])
            nc.sync.dma_start(out=wn_t[:], in_=wn_d[:])
            nc.sync.dma_start(out=bsage_t[:], in_=bsage_d[:])

            def out_block(src_fm, b, dram, base):
                """src_fm: [128 feat, 128 dst] SBUF -> transpose -> dram rows."""
                ps = tpsp.tile([128, 128], f32, tag="tp")
                nc.tensor.transpose(out=ps[:], in_=src_fm, identity=ident_t[:])
                orm = ormp.tile([128, 128], f32, tag="orm")
                nc.vector.tensor_copy(out=orm[:], in_=ps[:])
                nc.sync.dma_start(
                    out=dram[base + b * BLK : base + (b + 1) * BLK, :], in_=orm[:]
                )

            # ---- layer 0: xT = tanh(W_in.T @ h0T + b_in), write shard+table0
            H0G = 8
            h0piece = {}
            for b in range(NBLK):
                g, r = divmod(b, H0G)
                if r == 0:
                    nb = min(H0G, NBLK - g * H0G)
                    h0p = piecep.tile([DIN, H0G * BLK], f32, tag="h0p", name="h0p")
                    nc.sync.dma_start(
                        out=h0p[:, : nb * BLK],
                        in_=h0t[:, g * H0G * BLK : (g * H0G + nb) * BLK],
                    )
                    h0piece[g] = h0p
                ps = dnsp.tile([128, 128], f32, tag="dns")
                nc.tensor.matmul(
                    out=ps[:],
                    lhsT=win_t[:],
                    rhs=h0piece[g][:, r * BLK : (r + 1) * BLK],
                    start=True,
                    stop=True,
                )
                nc.scalar.activation(
                    out=xT[:, b * BLK : (b + 1) * BLK],
                    in_=ps[:],
                    func=mybir.ActivationFunctionType.Tanh,
                    bias=bin_t[:],
                )
                out_block(xT[:, b * BLK : (b + 1) * BLK], b, shard, 0)
            nc.gpsimd.collective_compute(
                "AllGather",
                mybir.AluOpType.bypass,
                ins=[shard[:]],
                outs=[tables[0][:]],
                replica_groups=rg,
            )

            # ---- GNN layers
            for l in range(NLAYER):
                table = tables[l]
                # aggregation: gather calls + mask matmuls in stream order
                piece_of_slot = {}
                for ci, (s, p0, n) in enumerate(calls):
                    ix = maskp.tile([128, CALL // 16], mybir.dt.int16, tag="ix", name="ix")
                    nc.sync.dma_start(
                        out=ix[:, : n // 16], in_=idxs[:, p0 // 16 : (p0 + n) // 16]
                    )
                    pc = piecep.tile([128, CALL // 128, 128], f32, tag="piece")
                    nc.gpsimd.dma_gather(
                        pc[:, : n // 128, :],
                        table[s * CH :, :],
                        ix[:, : n // 16],
                        n,
                        n,
                        D,
                        queue_num=ci % 4,
                    )
                    for t in range(p0 // 128, (p0 + n) // 128):
                        piece_of_slot[t] = (pc, t - p0 // 128)

                ps_cur = {}
                MG = 16
                mk_cur = None
                for i, (s, b, t, st, sp) in enumerate(entries):
                    pc, tl = piece_of_slot[t]
                    gi, ri = divmod(i, MG)
                    if ri == 0:
                        ng = min(MG, nent - gi * MG)
                        mk_cur = maskp.tile([128, MG, 128], f32, tag="mask", name="mk")
                        iota_b = bass.AP(
                            iota_t.tensor,
                            iota_t[:].offset,
                            [list(iota_t[:].ap[0]), [0, ng], list(iota_t[:].ap[1])],
                        )
                        dsl_b = dstrel_t[:, gi * MG : gi * MG + ng].to_broadcast(
                            [128, ng, 128]
                        )
                        w_b = wcol_t[:, gi * MG : gi * MG + ng].to_broadcast(
                            [128, ng, 128]
                        )
                        nc.vector.tensor_tensor(
                            out=mk_cur[:, :ng, :],
                            in0=iota_b,
                            in1=dsl_b,
                            op=mybir.AluOpType.is_equal,
                        )
                        nc.vector.tensor_tensor(
                            out=mk_cur[:, :ng, :],
                            in0=mk_cur[:, :ng, :],
                            in1=w_b,
                            op=mybir.AluOpType.mult,
                        )
                    if st:
                        ps_cur[b] = aggp.tile([128, 128], f32, tag="agg", name="aggps")
                    nc.tensor.matmul(
                        out=ps_cur[b][:],
                        lhsT=pc[:, tl, :],
                        rhs=mk_cur[:, ri, :],
                        start=st,
                        stop=sp,
                    )
                    if sp:
                        dsl = acc[:, b * BLK : (b + 1) * BLK]
                        if ev_first[b] == s:
                            nc.vector.tensor_copy(out=dsl, in_=ps_cur[b][:])
                        else:
                            nc.vector.tensor_add(out=dsl, in0=dsl, in1=ps_cur[b][:])

                # dense phase
                last = l == NLAYER - 1
                dram = out_d if last else shard
                for b in range(NBLK):
                    bsl = slice(b * BLK, (b + 1) * BLK)
                    ps = dnsp.tile([128, 128], f32, tag="dns")
                    nc.tensor.matmul(
                        out=ps[:],
                        lhsT=ws_t[:, l * D : (l + 1) * D],
                        rhs=xT[:, bsl],
                        start=True,
                        stop=False,
                    )
                    nc.tensor.matmul(
                        out=ps[:],
                        lhsT=wn_t[:, l * D : (l + 1) * D],
                        rhs=acc[:, bsl],
                        start=False,
                        stop=True,
                    )
                    nc.scalar.activation(
                        out=xT[:, bsl],
                        in_=ps[:],
                        func=mybir.ActivationFunctionType.Relu,
                        bias=bsage_t[:, l : l + 1],
                    )
                    out_block(xT[:, bsl], b, dram, 0)
                if not last:
                    nc.gpsimd.collective_compute(
                        "AllGather",
                        mybir.AluOpType.bypass,
                        ins=[shard[:]],
                        outs=[tables[l + 1][:]],
                        replica_groups=rg,
                    )

    nc.compile()
    return nc


def kernel(h0, src, dst, W_in, b_in, W_self, W_neigh, b_sage):
    from concourse.bass_utils import run_bass_kernel_spmd

    h0 = np.asarray(h0)
    src = np.asarray(src)
    dst = np.asarray(dst)
    key = "k"
    if key not in _CACHE:
        plan, data = _preprocess(src, dst)
        nc = _build(plan)
        _CACHE[key] = (plan, data, nc)
    plan, data, nc = _CACHE[key]
    perm = data["perm"]

    # permuted h0 (virtual rows zero), feature-major per core
    h0p = np.zeros((NT, DIN), np.float32)
    real = perm >= 0
    h0p[real] = h0[perm[real]]

    bin_col = np.zeros((128, 1), np.float32)
    bin_col[:D, 0] = b_in
    bsage_col = np.zeros((128, NLAYER), np.float32)
    bsage_col[:D, :] = np.asarray(b_sage).T
    iota = np.tile(np.arange(128, dtype=np.float32), (128, 1))
    ident = np.eye(128, dtype=np.float32)
    ws = np.concatenate([np.asarray(W_self)[l] for l in range(NLAYER)], axis=1).astype(np.float32)
    wn = np.concatenate([np.asarray(W_neigh)[l] for l in range(NLAYER)], axis=1).astype(np.float32)

    in_maps = []
    for c in range(NCORE):
        in_maps.append(
            {
                "h0t": np.ascontiguousarray(h0p[c * SH : (c + 1) * SH].T),
                "idxs": data["idxw"][c],
                "dstrel": data["dstrel"][c],
                "wcol": data["wcol"][c],
                "iota": iota,
                "ident": ident,
                "win": np.asarray(W_in, np.float32),
                "bin": bin_col,
                "ws": ws,
                "wn": wn,
                "bsage": bsage_col,
            }
        )

    global LAST_RESULT
    res = run_bass_kernel_spmd(
        nc, in_maps, core_ids=list(range(NCORE)), trace=TRACE
    )
    LAST_RESULT = res

    out = np.empty((N, D), np.float32)
    for c in range(NCORE):
        o = res.results[c]["out"]
        pc = perm[c * SH : (c + 1) * SH]
        m = pc >= 0
        out[pc[m]] = o[m]
    return out

